# revision 1
# baseline (speedup 1.0000x reference)
"""Trainium2 Bass kernel for nn_MetapopLayer (metapopulation SIR scan).

Math: per sample n (1024 total), M=64 locations, C=4 compartments, 100 steps:
    p[n,i]   = 1 - exp(sum_j log(1 - beta*rho[n,i,1]*Rt[n,i,j]/ntot[n,j]))
    q        = R @ p          (per-sample 64x64 matvec)
    new_inf  = (1 - sum_c rho) * q
    rho'     = rho @ T + e0*new_inf, clipped to [0, 1e10]
    trajectory records pre-update rho.

Key device trick: |beta*rho1*Rt/ntot| <= ~0.006, so
p(a) = 1 - exp(-sum_m a^m P_m/m)  (a = rho[n,i,1]) is replaced by a degree-D
polynomial  p(a) = sum_d c_d[n,i] a^d  with coefficients precomputed on host
in float64 (exact to ~1e-10, far below fp32 noise).  The device step is then
pure fp32 tensor ops: Horner (11 small DVE ops), a broadcast-mul + grouped
reduce for the matvec, and a broadcast-mul + grouped reduce for rho@T.

Sharding: pure data-parallel over samples; 128 samples per core on the 128
SBUF partitions.  Raw Bass (Block) implementation — the Tile context's tail
drain trips a sync-wait limit in this walrus build, so semaphores are manual.
"""
import numpy as np

import concourse.bass as bass
from concourse import mybir
from concourse.bass_utils import run_bass_kernel_spmd

F32 = mybir.dt.float32
N, M, C = 1024, 64, 4
TIMESTEPS = 100
NCORES = 8
NS = N // NCORES            # 128 samples per core = SBUF partitions
DEG = 6                     # polynomial degree for p(a)
CLIP_MAX = 1e10


# ----------------------------------------------------------------------
# host-side precompute: polynomial coefficients c_d[n,i]
# ----------------------------------------------------------------------
def _precompute_coeffs(R, beta):
    R64 = R.astype(np.float64)
    ntot = R64.sum(axis=1)                                   # (N, M)
    Rt = np.transpose(R64).reshape(N, M, M)                  # faithful reshape
    V = beta.astype(np.float64)[:, None, None] * Rt / ntot[:, None, :]

    DEG_I = 12   # internal composition degree
    # g(a) = sum_m (P_m/m) a^m
    G = np.zeros((DEG_I + 1, N, M))
    Vp = np.ones_like(V)
    for m in range(1, DEG_I + 1):
        Vp = Vp * V
        G[m] = Vp.sum(axis=2) / m
    # E = exp(-g) as truncated power series;  p = 1 - E
    E = np.zeros((DEG_I + 1, N, M))
    E[0] = 1.0
    Gj = np.zeros((DEG_I + 1, N, M)); Gj[0] = 1.0
    fact = 1.0
    for j in range(1, DEG_I + 1):
        new = np.zeros_like(Gj)
        for d1 in range(j - 1, DEG_I + 1):
            if not Gj[d1].any():
                continue
            for d2 in range(1, DEG_I + 1 - d1):
                new[d1 + d2] += Gj[d1] * G[d2]
        Gj = new
        fact *= j
        E += ((-1) ** j) * Gj / fact
    Cc = -E
    Cc[0] = 0.0
    return Cc[1 : DEG + 1].astype(np.float32)                # (DEG, N, M)


# ----------------------------------------------------------------------
# device kernel builder (per-core program, SPMD across 8 cores)
# ----------------------------------------------------------------------
def _build_bass(run_steps=TIMESTEPS):
    nc = bass.Bass()
    R_d = nc.dram_tensor("R", [NS, M * M], F32, kind="ExternalInput")     # (n,(i,k))
    cd_d = nc.dram_tensor("cd", [NS, DEG * M], F32, kind="ExternalInput")  # (n,(d,i))
    Tb_d = nc.dram_tensor("Tb", [NS, 16], F32, kind="ExternalInput")       # (n,(k,l))
    rho0_d = nc.dram_tensor("rho0", [NS, M * C], F32, kind="ExternalInput")
    traj_d = nc.dram_tensor("traj", [TIMESTEPS, NS, M * C], F32,
                            kind="ExternalOutput")

    mult, add_, mx = mybir.AluOpType.mult, mybir.AluOpType.add, mybir.AluOpType.max

    from contextlib import ExitStack
    with ExitStack() as ctx:
        R_t = ctx.enter_context(nc.sbuf_tensor("R_t", [NS, M * M], F32))
        cd_t = ctx.enter_context(nc.sbuf_tensor("cd_t", [NS, DEG * M], F32))
        Tb_t = ctx.enter_context(nc.sbuf_tensor("Tb_t", [NS, 16], F32))
        rhoA = ctx.enter_context(nc.sbuf_tensor("rhoA", [NS, M * C], F32))
        rhoB = ctx.enter_context(nc.sbuf_tensor("rhoB", [NS, M * C], F32))
        t_mv = ctx.enter_context(nc.sbuf_tensor("t_mv", [NS, M * M], F32))
        Gm = ctx.enter_context(nc.sbuf_tensor("Gm", [NS, M * 16], F32))
        h_t = ctx.enter_context(nc.sbuf_tensor("h_t", [NS, M], F32))
        p_t = ctx.enter_context(nc.sbuf_tensor("p_t", [NS, M], F32))
        q_t = ctx.enter_context(nc.sbuf_tensor("q_t", [NS, M], F32))
        sr_t = ctx.enter_context(nc.sbuf_tensor("sr_t", [NS, M], F32))
        u_t = ctx.enter_context(nc.sbuf_tensor("u_t", [NS, M], F32))
        ni_t = ctx.enter_context(nc.sbuf_tensor("ni_t", [NS, M], F32))
        ones_t = ctx.enter_context(nc.sbuf_tensor("ones_t", [NS, M], F32))
        zero_t = ctx.enter_context(nc.sbuf_tensor("zero_t", [NS, M], F32))
        s_in = ctx.enter_context(nc.semaphore("s_in"))
        s_state = ctx.enter_context(nc.semaphore("s_state"))
        s_out = ctx.enter_context(nc.semaphore("s_out"))
        s_gm = ctx.enter_context(nc.semaphore("s_gm"))
        block = ctx.enter_context(nc.Block())
        s_outB = ctx.enter_context(nc.semaphore("s_outB"))
        rho = [rhoA, rhoB]

        def rho_ap(buf, view):
            base = buf[:].ap[0]
            if view == "a":       # rho[:, 1::4]  (= compartment 1, per i)
                return bass.AP(buf, 1, [base, [4, M]])
            if view == "col0":    # rho[:, 0::4]
                return bass.AP(buf, 0, [base, [4, M]])
            if view == "ic":      # (i, c) for srho reduce
                return bass.AP(buf, 0, [base, [4, M], [1, 4]])
            if view == "G_in":    # (i, l, k): rho[n, i*4+k] bcast over l
                return bass.AP(buf, 0, [base, [4, M], [0, 4], [1, 4]])
            raise ValueError(view)

        @block.sync
        def _(sync):
            sync.dma_start(R_t[:], R_d[:, :]).then_inc(s_in, 16)
            sync.dma_start(cd_t[:], cd_d[:, :]).then_inc(s_in, 16)
            sync.dma_start(Tb_t[:], Tb_d[:, :]).then_inc(s_in, 16)
            sync.dma_start(rhoA[:], rho0_d[:, :]).then_inc(s_in, 16)
            sync.wait_ge(s_in, 64)                  # inputs landed
            H = M * C // 2
            for t in range(run_steps):
                sync.wait_ge(s_state, t)            # rho_t finalized
                dst = bass.AP(traj_d, t * NS * M * C,
                              [[M * C, NS], [1, H]])
                sync.dma_start(dst, rho[t % 2][:, 0:H]).then_inc(s_out, 16)
            sync.wait_ge(s_out, 16 * run_steps)     # all outputs landed
            sync.wait_ge(s_outB, 16 * run_steps)

        @block.scalar
        def _(scalar):
            H = M * C // 2
            scalar.wait_ge(s_in, 64)
            for t in range(run_steps):
                scalar.wait_ge(s_state, t)
                dst = bass.AP(traj_d, t * NS * M * C + H,
                              [[M * C, NS], [1, H]])
                scalar.dma_start(dst, rho[t % 2][:, H:]).then_inc(s_outB, 16)

        @block.gpsimd
        def _(gpsimd):
            # G-mul for step t: Gm[n,(i,l,k)] = rho_t[n,(i,k)] * T[n,(k,l)]
            Tb_bc = bass.AP(Tb_t, 0, [Tb_t[:].ap[0], [0, M], [1, 4], [4, 4]])
            Gm_v = Gm[:].rearrange("n (i l k) -> n i l k", i=M, l=4)
            gpsimd.wait_ge(s_in, 64)
            for t in range(run_steps):
                if t > 0:
                    gpsimd.wait_ge(s_state, t)      # rho_t ready + prev Gm read
                gpsimd.tensor_tensor(out=Gm_v, in0=rho_ap(rho[t % 2], "G_in"),
                                     in1=Tb_bc, op=mult).then_inc(s_gm, 1)

        @block.vector
        def _(vector):
            R_ik = R_t[:].rearrange("n (i k) -> n i k", i=M)
            t_ik = t_mv[:].rearrange("n (i k) -> n i k", i=M)
            p_bc = bass.AP(p_t, 0, [p_t[:].ap[0], [0, M], [1, M]])
            Gm_red = Gm[:].rearrange("n (il k) -> n il k", k=4)
            sub = mybir.AluOpType.subtract
            vector.memset(ones_t[:], 1.0)
            vector.memset(zero_t[:], 0.0)
            vector.wait_ge(s_in, 64)
            for t in range(run_steps):
                cur, nxt = rho[t % 2], rho[(t + 1) % 2]
                a_v = rho_ap(cur, "a")
                # srho, u = 1 - srho (early: consumed several ops later)
                vector.tensor_reduce(out=sr_t[:], in_=rho_ap(cur, "ic"),
                                     axis=mybir.AxisListType.X, op=add_)
                vector.tensor_tensor(out=u_t[:], in0=ones_t[:], in1=sr_t[:], op=sub)
                # p = Horner(c, a)
                vector.tensor_tensor(out=h_t[:], in0=cd_t[:, (DEG - 1) * M : DEG * M],
                                     in1=a_v, op=mult)
                for d in range(DEG - 1, 0, -1):
                    vector.tensor_tensor(out=h_t[:], in0=h_t[:],
                                         in1=cd_t[:, (d - 1) * M : d * M], op=add_)
                    if d > 1:
                        vector.tensor_tensor(out=h_t[:], in0=h_t[:], in1=a_v,
                                             op=mult)
                vector.tensor_tensor(out=p_t[:], in0=h_t[:], in1=a_v, op=mult)
                # q = R @ p  (broadcast-mul + grouped reduce)
                vector.tensor_tensor(out=t_ik, in0=R_ik, in1=p_bc, op=mult)
                vector.tensor_reduce(out=q_t[:], in_=t_ik,
                                     axis=mybir.AxisListType.X, op=add_)
                vector.tensor_tensor(out=ni_t[:], in0=u_t[:], in1=q_t[:], op=mult)
                # rho_next = rho @ T  (+ new_inf into c=0, clip)
                if t > 0:
                    vector.wait_ge(s_out, 16 * t)   # traj[t-1] DMA done
                    vector.wait_ge(s_outB, 16 * t)
                vector.wait_ge(s_gm, t + 1)         # Gm ready
                vector.tensor_reduce(out=nxt[:], in_=Gm_red,
                                     axis=mybir.AxisListType.X, op=add_)
                col0 = rho_ap(nxt, "col0")
                vector.tensor_tensor(out=col0, in0=col0, in1=ni_t[:], op=add_)
                vector.tensor_tensor(out=col0, in0=col0, in1=zero_t[:],
                                     op=mx).then_inc(s_state, 1)
    return nc


_NC_CACHE = None


def kernel(R, T, rho0, beta):
    global _NC_CACHE
    R = np.ascontiguousarray(R, np.float32)
    T = np.ascontiguousarray(T, np.float32)
    rho0 = np.ascontiguousarray(rho0, np.float32)
    beta = np.ascontiguousarray(beta, np.float32)

    cd = _precompute_coeffs(R, beta)                          # (DEG, N, M)
    cd_dev = np.ascontiguousarray(cd.transpose(1, 0, 2)).reshape(N, DEG * M)

    if _NC_CACHE is None:
        _NC_CACHE = _build_bass()
    nc = _NC_CACHE

    in_maps = []
    for c in range(NCORES):
        s = slice(c * NS, (c + 1) * NS)
        in_maps.append({
            "R": R[s].reshape(NS, M * M),
            "cd": cd_dev[s],
            "Tb": T[s].reshape(NS, 16),
            "rho0": rho0[s].reshape(NS, M * C),
        })
    res = run_bass_kernel_spmd(nc, in_maps, core_ids=list(range(NCORES)))
    parts = [r["traj"].reshape(TIMESTEPS, NS, M, C) for r in res.results]
    return np.concatenate(parts, axis=1)



# revision 2
# speedup vs baseline: 659.0633x; 659.0633x over previous
"""Trainium2 Bass kernel v2 for nn_MetapopLayer (metapopulation SIR scan).

Per sample n (1024), M=64 locations, C=4 compartments, 100 steps:
    p[n,i]   = 1 - exp(sum_j log(1 - beta*rho[n,i,1]*Rt[n,i,j]/ntot[n,j]))
    q        = R @ p          (per-sample 64x64 matvec)
    new_inf  = (1 - sum_c rho) * q
    rho'     = rho @ T + e0*new_inf            (clip is a provable no-op)
    trajectory records pre-update rho.

v2 design (vs v1: fp32 everywhere, 12.1us/step simulated):
  * p(a) as a degree-2 polynomial with host-exact (fp64) coefficients:
    p = (c2*a + c1)*a, fp16 on device. Norm-rel error of the whole
    pipeline vs the jax reference: 5.5e-4 (gate is 2e-2).
  * The 64x64 matvec in fp16: all-fp16 packed tensor_tensor ops run in
    the DVE 2x perf mode (0.52 ns/elem); the k-reduce is a pairwise add
    tree (fp16 stages, fp32 final) because tensor_reduce has no perf
    mode (1.04 ns/elem).
  * Mass conservation: sum_c(rho@T) = sum_c rho, so u = 1 - sum_c rho
    updates as u -= new_inf: no per-step reduce.
  * Self-clocking DVE pipeline: DVE computes rho' cols 1..3 itself
    (small slice of the rho@T reduce), so step t+1's a-read depends
    only on DVE program order. Pool concurrently does the big rho(x)T
    product (fp16 out), the q-tree tail, ni/u, col0-base and col0+=ni.
    Steady state: DVE ~5.2us busy/step, Pool ~4.5us, no ping-pong.
"""
import numpy as np

import concourse.bass as bass
from concourse import mybir
from concourse.bass_utils import run_bass_kernel_spmd

F32 = mybir.dt.float32
F16 = mybir.dt.float16
N, M, C = 1024, 64, 4
TIMESTEPS = 100
NCORES = 8
NS = N // NCORES            # 128 samples per core = SBUF partitions
DEG = 2                     # polynomial degree for p(a)


# ----------------------------------------------------------------------
# host-side precompute: polynomial coefficients c_d[n,i]
# ----------------------------------------------------------------------
def _precompute_coeffs(R, beta):
    R64 = R.astype(np.float64)
    ntot = R64.sum(axis=1)                                   # (N, M)
    Rt = np.transpose(R64).reshape(N, M, M)                  # faithful reshape
    V = beta.astype(np.float64)[:, None, None] * Rt / ntot[:, None, :]

    DEG_I = 12   # internal composition degree
    G = np.zeros((DEG_I + 1, N, M))
    Vp = np.ones_like(V)
    for m in range(1, DEG_I + 1):
        Vp = Vp * V
        G[m] = Vp.sum(axis=2) / m
    E = np.zeros((DEG_I + 1, N, M))
    E[0] = 1.0
    Gj = np.zeros((DEG_I + 1, N, M)); Gj[0] = 1.0
    fact = 1.0
    for j in range(1, DEG_I + 1):
        new = np.zeros_like(Gj)
        for d1 in range(j - 1, DEG_I + 1):
            if not Gj[d1].any():
                continue
            for d2 in range(1, DEG_I + 1 - d1):
                new[d1 + d2] += Gj[d1] * G[d2]
        Gj = new
        fact *= j
        E += ((-1) ** j) * Gj / fact
    Cc = -E
    Cc[0] = 0.0
    return Cc[1 : DEG + 1]                                   # (DEG, N, M) f64


# ----------------------------------------------------------------------
# device kernel builder (per-core program, SPMD across 8 cores)
# ----------------------------------------------------------------------
def _build_bass(run_steps=TIMESTEPS):
    nc = bass.Bass()
    R_d = nc.dram_tensor("R16", [NS, M * M], F16, kind="ExternalInput")    # (n,(i,k))
    c1_d = nc.dram_tensor("c1", [NS, M], F16, kind="ExternalInput")
    c2_d = nc.dram_tensor("c2", [NS, M], F16, kind="ExternalInput")
    Tb_d = nc.dram_tensor("Tb", [NS, 16], F32, kind="ExternalInput")       # (n,(k,l))
    rho0_d = nc.dram_tensor("rho0", [NS, M * C], F32, kind="ExternalInput")
    u0_d = nc.dram_tensor("u0", [NS, M], F32, kind="ExternalInput")
    traj_d = nc.dram_tensor("traj", [TIMESTEPS, NS, M * C], F32,
                            kind="ExternalOutput")

    mult, add_ = mybir.AluOpType.mult, mybir.AluOpType.add
    sub = mybir.AluOpType.subtract

    from contextlib import ExitStack
    with ExitStack() as ctx:
        R_t = ctx.enter_context(nc.sbuf_tensor("R_t", [NS, M * M], F16))
        c1_t = ctx.enter_context(nc.sbuf_tensor("c1_t", [NS, M], F16))
        c2_t = ctx.enter_context(nc.sbuf_tensor("c2_t", [NS, M], F16))
        Tb_t = ctx.enter_context(nc.sbuf_tensor("Tb_t", [NS, 16], F32))
        rhoA = ctx.enter_context(nc.sbuf_tensor("rhoA", [NS, M * C], F32))
        rhoB = ctx.enter_context(nc.sbuf_tensor("rhoB", [NS, M * C], F32))
        u_t = ctx.enter_context(nc.sbuf_tensor("u_t", [NS, M], F32))
        m_t = ctx.enter_context(nc.sbuf_tensor("m_t", [NS, M], F16))
        p_t = ctx.enter_context(nc.sbuf_tensor("p_t", [NS, M], F16))
        tm = ctx.enter_context(nc.sbuf_tensor("tm", [NS, M * M], F16))      # i,k
        t1_t = ctx.enter_context(nc.sbuf_tensor("t1_t", [NS, M * 32], F16))
        t2_t = ctx.enter_context(nc.sbuf_tensor("t2_t", [NS, M * 16], F16))
        t3_t = ctx.enter_context(nc.sbuf_tensor("t3_t", [NS, M * 8], F16))
        t4_t = ctx.enter_context(nc.sbuf_tensor("t4_t", [NS, M * 4], F16))
        t5_t = ctx.enter_context(nc.sbuf_tensor("t5_t", [NS, M * 2], F16))
        q_t = ctx.enter_context(nc.sbuf_tensor("q_t", [NS, M], F32))
        ni_t = ctx.enter_context(nc.sbuf_tensor("ni_t", [NS, M], F32))
        Gm = ctx.enter_context(nc.sbuf_tensor("Gm", [NS, M * 16], F16))     # i,l,k
        G0_t = ctx.enter_context(nc.sbuf_tensor("G0_t", [NS, M * 2], F16))  # l=0 pairs
        G123 = ctx.enter_context(nc.sbuf_tensor("G123", [NS, M * 6], F16))  # l=1..3
        s_in = ctx.enter_context(nc.semaphore("s_in"))
        s_gmm = ctx.enter_context(nc.semaphore("s_gmm"))   # Pool Gm-mult done
        s_t3 = ctx.enter_context(nc.semaphore("s_t3"))     # DVE tree t3 done
        s_t4 = ctx.enter_context(nc.semaphore("s_t4"))     # Pool t4 done (t3 free)
        s_st = ctx.enter_context(nc.semaphore("s_st"))     # Pool col0 done
        s_gmr = ctx.enter_context(nc.semaphore("s_gmr"))   # DVE s2l123 done
        s_out = ctx.enter_context(nc.semaphore("s_out"))   # traj[t] DMA done
        block = ctx.enter_context(nc.Block())
        rho = [rhoA, rhoB]

        def a_ap(buf):      # rho[:, 1::4] — compartment 1 per location
            return bass.AP(buf, 1, [buf[:].ap[0], [4, M]])

        def col0_ap(buf):
            return bass.AP(buf, 0, [buf[:].ap[0], [4, M]])

        def halves(buf, w):
            """Pairwise-tree inputs over a contiguous (n,(i,w)) buffer."""
            base = buf[:].ap[0]
            h = w // 2
            if h == 1:
                return (bass.AP(buf, 0, [base, [w, M]]),
                        bass.AP(buf, 1, [base, [w, M]]))
            return (bass.AP(buf, 0, [base, [w, M], [1, h]]),
                    bass.AP(buf, h, [base, [w, M], [1, h]]))

        def out3(buf, w):
            return bass.AP(buf, 0, [buf[:].ap[0], [w, M], [1, w]])

        # ---------------- DMA queue (sync engine) ----------------
        @block.sync
        def _(sync):
            sync.dma_start(R_t[:], R_d[:, :]).then_inc(s_in, 16)
            sync.dma_start(c1_t[:], c1_d[:, :]).then_inc(s_in, 16)
            sync.dma_start(c2_t[:], c2_d[:, :]).then_inc(s_in, 16)
            sync.dma_start(Tb_t[:], Tb_d[:, :]).then_inc(s_in, 16)
            sync.dma_start(rhoA[:], rho0_d[:, :]).then_inc(s_in, 16)
            sync.dma_start(u_t[:], u0_d[:, :]).then_inc(s_in, 16)
            for t in range(run_steps):
                if t > 0:
                    sync.wait_ge(s_st, t)           # col0 of rho_t done (Pool)
                    sync.wait_ge(s_gmr, t)          # cols 1..3 of rho_t done (DVE)
                else:
                    sync.wait_ge(s_in, 96)
                dst = bass.AP(traj_d, t * NS * M * C,
                              [[M * C, NS], [1, M * C]])
                sync.dma_start(dst, rho[t % 2][:, :]).then_inc(s_out, 16)
            sync.wait_ge(s_out, 16 * run_steps)

        # ---------------- DVE: p-chain, matvec mult + tree head,
        #                  rho' cols 1..3 (self-clocking a-path) --------
        @block.vector
        def _(vector):
            R_ik = R_t[:].rearrange("n (i k) -> n i k", i=M)
            tm_ik = tm[:].rearrange("n (i k) -> n i k", i=M)
            p_bc = bass.AP(p_t, 0, [p_t[:].ap[0], [0, M], [1, M]])
            gb = Gm[:].ap[0]
            vector.wait_ge(s_in, 96)
            for t in range(run_steps - 1):
                cur, nxt = rho[t % 2], rho[(t + 1) % 2]
                a_v = a_ap(cur)
                # p = (c2*a + c1) * a   (fp16; a read fp32-strided)
                vector.tensor_tensor(out=m_t[:], in0=c2_t[:], in1=a_v, op=mult)
                vector.tensor_tensor(out=m_t[:], in0=m_t[:], in1=c1_t[:], op=add_)
                vector.tensor_tensor(out=p_t[:], in0=m_t[:], in1=a_v, op=mult)
                # t_mv = R * p  (fp16 2x)
                vector.tensor_tensor(out=tm_ik, in0=R_ik, in1=p_bc, op=mult)
                # pairwise tree head: 64 -> 32 -> 16 -> 8
                i0, i1 = halves(tm, 64)
                vector.tensor_tensor(out=out3(t1_t, 32), in0=i0, in1=i1, op=add_)
                i0, i1 = halves(t1_t, 32)
                vector.tensor_tensor(out=out3(t2_t, 16), in0=i0, in1=i1, op=add_)
                if t > 0:
                    vector.wait_ge(s_t4, t)         # Pool consumed t3 of t-1
                i0, i1 = halves(t2_t, 16)
                vector.tensor_tensor(out=out3(t3_t, 8), in0=i0, in1=i1,
                                     op=add_).then_inc(s_t3, 1)
                # rho_{t+1} cols 1..3 = (rho_t @ T)[:, 1:4] from Pool's Gm
                vector.wait_ge(s_gmm, t + 1)
                if t > 0:
                    vector.wait_ge(s_out, 16 * t)   # nxt buffer free (DMA t-1)
                g0 = bass.AP(Gm, 4, [gb, [16, M], [4, 3], [1, 2]])
                g1 = bass.AP(Gm, 6, [gb, [16, M], [4, 3], [1, 2]])
                go = bass.AP(G123, 0, [G123[:].ap[0], [6, M], [2, 3], [1, 2]])
                vector.tensor_tensor(out=go, in0=g0, in1=g1, op=add_)
                gg0 = bass.AP(G123, 0, [G123[:].ap[0], [6, M], [2, 3]])
                gg1 = bass.AP(G123, 1, [G123[:].ap[0], [6, M], [2, 3]])
                no = bass.AP(nxt, 1, [nxt[:].ap[0], [4, M], [1, 3]])
                vector.tensor_tensor(out=no, in0=gg0, in1=gg1,
                                     op=add_).then_inc(s_gmr, 1)

        # ---------------- Pool: Gm mult, q-tree tail, ni/u, col0 ---------
        @block.gpsimd
        def _(gpsimd):
            Tb_bc = bass.AP(Tb_t, 0, [Tb_t[:].ap[0], [0, M], [1, 4], [4, 4]])
            Gm_v = Gm[:].rearrange("n (i l k) -> n i l k", i=M, l=4)
            gb = Gm[:].ap[0]
            gpsimd.wait_ge(s_in, 96)
            for t in range(run_steps - 1):
                cur, nxt = rho[t % 2], rho[(t + 1) % 2]
                if t > 0:
                    gpsimd.wait_ge(s_gmr, t)        # DVE done reading Gm of t-1
                                                    # (+ cols 1..3 of rho_t written)
                # Gm[n,(i,l,k)] = rho_t[n,(i,k)] * T[n,(k,l)]  (fp16 out)
                rho_in = bass.AP(cur, 0, [cur[:].ap[0], [4, M], [0, 4], [1, 4]])
                gpsimd.tensor_tensor(out=Gm_v, in0=rho_in, in1=Tb_bc,
                                     op=mult).then_inc(s_gmm, 1)
                # rho_{t+1} col0 base = (rho_t @ T)[:, 0]
                g0 = bass.AP(Gm, 0, [gb, [16, M], [1, 2]])
                g1 = bass.AP(Gm, 2, [gb, [16, M], [1, 2]])
                gpsimd.tensor_tensor(out=out3(G0_t, 2), in0=g0, in1=g1, op=add_)
                if t > 0:
                    gpsimd.wait_ge(s_out, 16 * t)   # nxt buffer free (DMA t-1)
                i0, i1 = halves(G0_t, 2)
                gpsimd.tensor_tensor(out=col0_ap(nxt), in0=i0, in1=i1, op=add_)
                # q-tree tail: 8 -> 4 -> 2 -> 1 (fp32 final)
                gpsimd.wait_ge(s_t3, t + 1)
                i0, i1 = halves(t3_t, 8)
                gpsimd.tensor_tensor(out=out3(t4_t, 4), in0=i0, in1=i1,
                                     op=add_).then_inc(s_t4, 1)
                i0, i1 = halves(t4_t, 4)
                gpsimd.tensor_tensor(out=out3(t5_t, 2), in0=i0, in1=i1, op=add_)
                i0, i1 = halves(t5_t, 2)
                gpsimd.tensor_tensor(out=q_t[:], in0=i0, in1=i1, op=add_)
                # ni = u*q ; u -= ni ; col0 += ni
                gpsimd.tensor_tensor(out=ni_t[:], in0=u_t[:], in1=q_t[:], op=mult)
                gpsimd.tensor_tensor(out=u_t[:], in0=u_t[:], in1=ni_t[:], op=sub)
                c0 = col0_ap(nxt)
                gpsimd.tensor_tensor(out=c0, in0=c0, in1=ni_t[:],
                                     op=add_).then_inc(s_st, 1)
    return nc


_NC_CACHE = None


def kernel(R, T, rho0, beta):
    global _NC_CACHE
    R = np.ascontiguousarray(R, np.float32)
    T = np.ascontiguousarray(T, np.float32)
    rho0 = np.ascontiguousarray(rho0, np.float32)
    beta = np.ascontiguousarray(beta, np.float32)

    cd = _precompute_coeffs(R, beta)                          # (DEG, N, M) f64
    c1 = cd[0].astype(np.float16)
    c2 = cd[1].astype(np.float16)
    R16 = R.reshape(N, M * M).astype(np.float16)
    u0 = (1.0 - rho0.sum(axis=2)).astype(np.float32)          # (N, M)

    if _NC_CACHE is None:
        _NC_CACHE = _build_bass()
    nc = _NC_CACHE

    in_maps = []
    for c in range(NCORES):
        s = slice(c * NS, (c + 1) * NS)
        in_maps.append({
            "R16": R16[s],
            "c1": c1[s],
            "c2": c2[s],
            "Tb": T[s].reshape(NS, 16),
            "rho0": rho0[s].reshape(NS, M * C),
            "u0": u0[s],
        })
    res = run_bass_kernel_spmd(nc, in_maps, core_ids=list(range(NCORES)))
    parts = [r["traj"].reshape(TIMESTEPS, NS, M, C) for r in res.results]
    return np.concatenate(parts, axis=1)
